# revision 6
# baseline (speedup 1.0000x reference)
"""MeshUnPool gather kernel for 8 Trainium2 NeuronCores.

reference: out[i, :] = features[parent_idx[i], :]
  features: [500000, 256] f32 (512 MB), parent_idx: [1000000] int64/int32,
  out: [1000000, 256] f32 (1 GB).

Strategy (per core, 125952 rows): two-hop bf16 row movement through SBUF
using the SWDGE gather/scatter ucode (0.34 ns/descriptor desc-gen, vs
~8.5 ns/row for the indirect1d path), spread over all 4 SWDGE queues:

  1. dma_gather pieces (<=1024 rows, the SWDGE ring cap) pull rows from
     the table into SBUF tiles. int16 gather indices address a stride-16
     window (elem_step = 16 rows), so 16 residue classes cover the full
     500K-row table.
  2. dma_scatter_add pieces push each SBUF slot to its output row inside
     a per-supertile 31488-row window (4 supertiles/core). Scatter only
     ADDs, so the output slab is zeroed first by fat HWDGE stores
     (overlapped per supertile). Pad slots land on per-supertile trash
     rows that the host strips.

Rows are bucketed host-side by (supertile, table_row % 16); bucket
capacities take the max over the 8 cores so all cores share one NEFF
(the program is input-specialized and compiled on first call).
bf16 halves traffic; max rel err 2^-9 ~ 2e-3, well under the 2e-2 gate
(fp16 would fail near-zero values vs the 1e-6-clamped denominator).
The gather->scatter emission uses a lag-16 software pipeline so scatter
semaphore waits never stall the in-order GPSIMD engine.
"""

import numpy as np
import ml_dtypes

import concourse.bass as bass
import concourse.bacc as bacc
import concourse.mybir as mybir
import concourse.tile as tile
from concourse.bass import AP
from concourse.bass_utils import run_bass_kernel_spmd

N_POOLED = 500000
N_UNPOOLED = 1000000
C = 256
NCORES = 8
P = 128
BF = ml_dtypes.bfloat16

ROWS_PER_CORE = 125952            # 8 * 125952 = 1007616 (0.76% pad)
NST = 4
ST = ROWS_PER_CORE // NST         # 31488 rows per supertile
TRASH = 16
OUTW = ST + TRASH                 # 31504-row device window per supertile
GSTRIDE = 16                      # gather window stride (rows)
GWIN = 31250                      # gather window row count (16*31249+15 < 500000)
NGC = 16                          # gather residue classes
PIECE = 1024                      # SWDGE ring cap (descs per instruction)
LAG = 16
NTILE = 32
ZCOLS = 41                        # zero-store tile cols

_cache = {}


def _plan(idx32):
    """Bucket rows by (supertile, table_row % 16); uniform piece geometry
    across cores (max count per bucket). Returns piece list + per-core
    gather/scatter idx arrays."""
    a = idx32.reshape(NCORES, ROWS_PER_CORE)
    within = np.arange(ROWS_PER_CORE)
    st_of = within // ST
    counts = np.zeros((NCORES, NST, NGC), dtype=np.int64)
    orders = []
    for c in range(NCORES):
        gcls = a[c] % GSTRIDE
        key = st_of * NGC + gcls
        order = np.argsort(key, kind="stable")
        orders.append(order)
        cnt = np.bincount(key, minlength=NST * NGC)
        counts[c] = cnt.reshape(NST, NGC)
    maxn = counts.max(axis=0)                       # [NST, NGC]
    cols_b = -(-maxn // P)                          # cols per bucket

    # piece list: (st, gclass, piece_cols), pieces of <= 8 cols
    pieces = []
    for st in range(NST):
        for g in range(NGC):
            left = int(cols_b[st, g])
            while left > 0:
                k = min(8, left)
                pieces.append((st, g, k))
                left -= k
    tot_cols = sum(k for _, _, k in pieces)

    # per-core idx arrays
    gidx = np.zeros((NCORES, 128, tot_cols * 8), dtype=np.int16)
    sidx = np.zeros((NCORES, 128, tot_cols * 8), dtype=np.int16)
    for c in range(NCORES):
        order = orders[c]
        rows_sorted = order                          # out row (core-local)
        tbl_sorted = a[c][order]
        # bucket start offsets in sorted order
        bstart = np.zeros(NST * NGC + 1, dtype=np.int64)
        np.cumsum(counts[c].reshape(-1), out=bstart[1:])
        cur = {}
        col_off = 0
        for (st, g, k) in pieces:
            b = st * NGC + g
            pos = cur.get(b, 0)
            n_b = counts[c, st, g]
            take = min(k * P, n_b - pos)
            sl = slice(bstart[b] + pos, bstart[b] + pos + take)
            gl = np.zeros(k * P, dtype=np.int16)
            slc = np.full(k * P, ST, dtype=np.int16)  # trash row (local)
            if take > 0:
                gl[:take] = ((tbl_sorted[sl] - g) // GSTRIDE).astype(np.int16)
                slc[:take] = (rows_sorted[sl] - st * ST).astype(np.int16)
            cur[b] = pos + take
            w16 = gl.reshape(k * 8, 16).T
            gidx[c, :, col_off:col_off + k * 8] = np.tile(w16, (8, 1))
            w16 = slc.reshape(k * 8, 16).T
            sidx[c, :, col_off:col_off + k * 8] = np.tile(w16, (8, 1))
            col_off += k * 8
    return pieces, tot_cols, gidx, sidx


def _build(pieces, tot_cols):
    nc = bacc.Bacc("TRN2", target_bir_lowering=False, debug=False,
                   num_devices=NCORES, num_swdge_queues=4)
    feat = nc.dram_tensor("features", [N_POOLED, C], mybir.dt.bfloat16,
                          kind="ExternalInput").ap()
    gidx = nc.dram_tensor("gidx", [128, tot_cols * 8], mybir.dt.int16,
                          kind="ExternalInput").ap()
    sidx = nc.dram_tensor("sidx", [128, tot_cols * 8], mybir.dt.int16,
                          kind="ExternalInput").ap()
    out = nc.dram_tensor("out", [NST * OUTW, C], mybir.dt.bfloat16,
                         kind="ExternalOutput").ap()

    def g_window(g):
        a = feat
        return AP(a.tensor, g * C, [[GSTRIDE * C, GWIN], [1, C]])

    def s_window(st):
        a = out
        return AP(a.tensor, st * OUTW * C, [[C, ST + 1], [1, C]])

    with tile.TileContext(nc) as tc:
        with tc.tile_pool(name="g", bufs=1) as gp, \
             tc.tile_pool(name="i", bufs=1) as ip, \
             tc.tile_pool(name="z", bufs=1) as zp:
            git = ip.tile([128, tot_cols * 8], mybir.dt.int16)
            sit = ip.tile([128, tot_cols * 8], mybir.dt.int16)
            nc.scalar.dma_start(out=git[:], in_=gidx[:])
            nc.scalar.dma_start(out=sit[:], in_=sidx[:])
            # zero the real output rows (trash rows stay garbage)
            zt = zp.tile([P, ZCOLS * C], mybir.dt.bfloat16)
            nc.vector.memset(zt[:], 0.0)
            for st in range(NST):
                zrows = ST // (P * ZCOLS)            # 6 stores of 41 cols
                for z in range(zrows):
                    eng = nc.sync if z % 2 == 0 else nc.scalar
                    base = st * OUTW + z * P * ZCOLS
                    eng.dma_start(
                        out=out[base:base + P * ZCOLS, :].rearrange(
                            "(p t) c -> p t c", p=P),
                        in_=zt[:].rearrange("p (t c) -> p t c", c=C))

            col_off = [0] * (len(pieces) + 1)
            for j, (_, _, k) in enumerate(pieces):
                col_off[j + 1] = col_off[j] + k
            tiles = {}
            npieces = len(pieces)
            ndma = 0                 # Pool-DMA emission counter: queue must
                                     # track the 8-lane DMASW sem rotation
            for j in range(npieces + LAG):
                if j < npieces:
                    st, g, k = pieces[j]
                    gt = gp.tile([P, 8 * C], mybir.dt.bfloat16,
                                 name=f"gt{j % NTILE}")
                    tiles[j] = gt
                    io = col_off[j] * 8
                    nc.gpsimd.dma_gather(
                        out_ap=gt[:, :k * C].rearrange("p (t c) -> p t c", c=C),
                        in_ap=g_window(g),
                        idxs_ap=git[:, io:io + k * 8],
                        num_idxs=k * P, num_idxs_reg=k * P, elem_size=C,
                        elem_step=GSTRIDE * C, queue_num=ndma % 4)
                    ndma += 1
                if j >= LAG:
                    i = j - LAG
                    st, g, k = pieces[i]
                    gt = tiles.pop(i)
                    io = col_off[i] * 8
                    nc.gpsimd.dma_scatter_add(
                        out_ap=s_window(st),
                        in_ap=gt[:, :k * C].rearrange("p (t c) -> p t c", c=C),
                        idxs_ap=sit[:, io:io + k * 8],
                        num_idxs=k * P, num_idxs_reg=k * P, elem_size=C,
                        queue_num=ndma % 4)
                    ndma += 1
    # Align each SWDGE queue with its Tile-assigned DMASW sem lane
    # (lane L may only ever update from one queue): queue := lane % 4.
    _align_queues(nc)
    nc.compile()
    return nc


def _align_queues(nc):
    DMASW0 = 11
    n = 0
    for inst in nc.inst_map.values():
        if isinstance(inst, (mybir.InstDMAGatherAnt, mybir.InstDMAScatterAddAnt)):
            proc = inst.bass_scheduled_proc
            assert proc is not None and DMASW0 <= proc < DMASW0 + 8, (
                f"{inst.name}: proc={proc}"
            )
            inst.queue_num = (proc - DMASW0) % 4
            n += 1
    assert n > 0


def _run(features, parent_idx, **spmd_kwargs):
    feat = np.ascontiguousarray(
        np.asarray(features, dtype=np.float32).astype(BF))
    idx32 = np.zeros(ROWS_PER_CORE * NCORES, dtype=np.int32)
    idx32[:N_UNPOOLED] = np.asarray(parent_idx).astype(np.int32)

    pieces, tot_cols, gidx, sidx = _plan(idx32)
    key = ("gs", tot_cols, tuple(k for _, _, k in pieces))
    if key not in _cache:
        _cache.clear()
        _cache[key] = _build(pieces, tot_cols)
    nc = _cache[key]

    in_maps = [{"features": feat,
                "gidx": np.ascontiguousarray(gidx[c]),
                "sidx": np.ascontiguousarray(sidx[c])}
               for c in range(NCORES)]
    res = run_bass_kernel_spmd(nc, in_maps, core_ids=list(range(NCORES)),
                               **spmd_kwargs)
    outs = []
    for c in range(NCORES):
        o = np.asarray(res.results[c]["out"]).reshape(NST, OUTW, C)
        outs.append(o[:, :ST, :].reshape(ROWS_PER_CORE, C))
    out = np.concatenate(outs, axis=0)[:N_UNPOOLED]
    return out.astype(np.float32), res


def kernel(features, parent_idx):
    out, _ = _run(features, parent_idx)
    return out


# revision 7
# speedup vs baseline: 1.9526x; 1.9526x over previous
"""MeshUnPool gather kernel for 8 Trainium2 NeuronCores.

reference: out[i, :] = features[parent_idx[i], :]
  features: [500000, 256] f32 (512 MB), parent_idx: [1000000] int64/int32,
  out: [1000000, 256] f32 (1 GB).

Sharding: output rows sharded across the 8 cores; feature table replicated.

Per core (125952 rows): indirect row-gather DMAs (128 rows/instruction --
the SWDGE ucode consumes one int32 index per SBUF partition) pull random
1KB table rows into SBUF. Rows are assigned p-major (gather (b,j) covers
rows b*3072 + p*24 + j), so each partition's 24 rows per block are
CONTIGUOUS in the output and the store flushes as fat 24KB descriptors,
alternating across both HWDGE queues (sync/scalar). The baseline wrote
1KB store descriptors through one HWDGE queue (~10.7 ns each, 1.35 ms);
fat descriptors collapse the store side so the wall is the GPSIMD
desc-gen floor (984 instructions x ~1.09 us). Output is bit-exact.
"""

import numpy as np

import concourse.bass as bass
import concourse.bacc as bacc
import concourse.mybir as mybir
import concourse.tile as tile
from concourse.bass_utils import run_bass_kernel_spmd

N_POOLED = 500000
N_UNPOOLED = 1000000
C = 256
NCORES = 8
P = 128

# rows per core = P * GPB * NB ; 8 * 125952 = 1007616 (0.76% pad)
GPB = 24          # gathers (128 rows each) per store block
NB = 41           # store blocks per core
ROWS_PER_CORE = P * GPB * NB

_cache = {}


def _build():
    nc = bacc.Bacc("TRN2", target_bir_lowering=False, debug=False,
                   num_devices=NCORES)
    feat = nc.dram_tensor("features", [N_POOLED, C], mybir.dt.float32,
                          kind="ExternalInput").ap()
    # host ships idx p-major: element (p, b*GPB+j) = idx[b*3072 + p*GPB + j]
    idx = nc.dram_tensor("parent_idx", [P, GPB * NB], mybir.dt.int32,
                         kind="ExternalInput").ap()
    out = nc.dram_tensor("out", [ROWS_PER_CORE, C], mybir.dt.float32,
                         kind="ExternalOutput").ap()

    with tile.TileContext(nc) as tc:
        with tc.tile_pool(name="g", bufs=3) as gp, \
             tc.tile_pool(name="i", bufs=1) as ip:
            idx_tile = ip.tile([P, GPB * NB], mybir.dt.int32)
            nc.scalar.dma_start(out=idx_tile[:], in_=idx[:])
            for b in range(NB):
                gtile = gp.tile([P, GPB * C], mybir.dt.float32)
                for j in range(GPB):
                    t = b * GPB + j
                    nc.gpsimd.indirect_dma_start(
                        out=gtile[:, j * C:(j + 1) * C],
                        out_offset=None,
                        in_=feat[:],
                        in_offset=bass.IndirectOffsetOnAxis(
                            ap=idx_tile[:, t:t + 1], axis=0),
                    )
                # block rows p-major: row b*3072 + p*GPB + j = gtile[p, j]
                eng = nc.sync if b % 2 == 0 else nc.scalar
                eng.dma_start(
                    out=out[b * GPB * P:(b + 1) * GPB * P, :].rearrange(
                        "(p j) c -> p j c", p=P),
                    in_=gtile[:].rearrange("p (j c) -> p j c", c=C),
                )
    nc.compile()
    return nc


def _run(features, parent_idx, **spmd_kwargs):
    feat = np.ascontiguousarray(np.asarray(features), dtype=np.float32)
    idx32 = np.zeros(ROWS_PER_CORE * NCORES, dtype=np.int32)
    idx32[:N_UNPOOLED] = np.asarray(parent_idx).astype(np.int32)
    # per core: row b*128*GPB + p*GPB + j  ->  idx element (p, b*GPB + j)
    shards = (idx32.reshape(NCORES, NB, P, GPB)
              .transpose(0, 2, 1, 3).reshape(NCORES, P, NB * GPB))

    if "nc" not in _cache:
        _cache["nc"] = _build()
    nc = _cache["nc"]

    in_maps = [{"features": feat,
                "parent_idx": np.ascontiguousarray(shards[c])}
               for c in range(NCORES)]
    res = run_bass_kernel_spmd(nc, in_maps, core_ids=list(range(NCORES)),
                               **spmd_kwargs)
    out = np.concatenate([r["out"] for r in res.results], axis=0)[:N_UNPOOLED]
    return out, res


def kernel(features, parent_idx):
    out, _ = _run(features, parent_idx)
    return out
